# revision 22
# baseline (speedup 1.0000x reference)
"""Trainium2 Bass kernel for causal GQA self-attention (S=2048, D=4096, H=32,
HKV=8, DH=128), tensor-parallel over 8 NeuronCores.

Sharding: head-parallel TP. Core i owns q-heads [4i..4i+4) and kv-head i:
  - qkv_proj column shard -> q [S,512], k [S,128], v [S,128]
  - RoPE + causal attention for its 4 heads (GQA group shares the kv head)
  - o_proj row shard (rows [512i..512i+512)) -> bf16 partial [S, D]
Host sums the 8 partials (the "all-reduce") and reshapes to [S, 1, D].

v2 design ("S^T-direct"): attention scores are computed directly in kv-major
layout, sT[kv, q] = K·Q^T, using the dh-major K/Q slabs that the projection
already produces — this removes all 544 PE transposes of P and the 544
PSUM->SBUF copies that made the v1 pipeline DVE-bound and let HAM oscillate.
The softmax denominator l[q] = sum_kv exp(s) is computed on the tensor engine
with an all-ones [128,128] stationary operand: same N-stream cost as any MM,
and the result lands replicated on all 128 partitions, so the per-q reciprocal
can be applied to ctx^T with a single DVE multiply (no partition broadcast).

Per-core phases, interleaved per 512-row sequence chunk n (PE stays dense):
  1. qkv projection, 6 slabs of 128 cols (4q + k + v), N=512 streams;
     q/k slabs RoPE'd in place, v slab PE-transposed to seq-major tiles.
  2. attention for q-chunk c=n: per kv-tile t: sT MM (causal suffix only) ->
     diag mask add (DVE) -> exp (scalar, direct to SBUF bf16) -> l MM + PV MM,
     software-pipelined by 2 tiles so the PE never waits on the scalar exp.
  3. o_proj row shard for chunk c=n, N=512 streams, bf16 partial out.

All matmuls run in bf16 with fp32 PSUM accumulation. Softmax runs without
max-subtraction (logits are O(10) here, far inside fp32 exp range).
"""

import sys

sys.path.insert(0, "/opt/trn_rl_repo")

import numpy as np
import ml_dtypes
from contextlib import ExitStack

import concourse.bass as bass
import concourse.tile as tile
from concourse import mybir
from concourse.bass_utils import run_bass_kernel_spmd
from concourse.masks import make_identity

S, B, D = 2048, 1, 4096
H, HKV, DH = 32, 8, 128
NCORES = 8
HQ = H // HKV  # q heads per core = 4
NSLAB = HQ + 2  # 4 q slabs + k + v
THETA = 10000.0
SCALE = 1.0 / float(np.sqrt(DH))

BF16 = mybir.dt.bfloat16
F32 = mybir.dt.float32
np_bf16 = ml_dtypes.bfloat16

NKB = D // 128  # 32 contraction blocks for the projections
NQB = S // 128  # 16 seq blocks of 128
NCHUNK = S // 512  # 4 sequence chunks of 512


def build_kernel() -> bass.Bass:
    nc = bass.Bass()

    # hidT packed host-side as contiguous [chunk, kb, 128, 512] tiles so each
    # DMA is one dense 128KB block (strided 1KB rows run at ~half DMA rate)
    hidT_e = nc.declare_dram_parameter(
        "hidT", [NCHUNK, NKB, 128, 512], BF16, isOutput=False
    )
    wqkv_e = nc.declare_dram_parameter("wqkv", [D, NSLAB * DH], BF16, isOutput=False)
    wo_e = nc.declare_dram_parameter("wo", [HQ * DH, D], BF16, isOutput=False)
    # cos2 = [cos; cos], sinS = [-sin; sin]  (dh-major halves stacked)
    cos_e = nc.declare_dram_parameter("cos2", [128, S], BF16, isOutput=False)
    sin_e = nc.declare_dram_parameter("sinS", [128, S], BF16, isOutput=False)
    # out packed [qb, dc, 128, 512]; host unpacks to [S, D]
    out_e = nc.declare_dram_parameter("out", [NQB, 8, 128, 512], BF16, isOutput=True)

    hidT = hidT_e[:]
    wqkv = wqkv_e[:]
    wo = wo_e[:]
    out = out_e[:]

    with tile.TileContext(nc) as tc, ExitStack() as ctx:
        singles = ctx.enter_context(tc.tile_pool(name="singles", bufs=1))

        # ---- persistent SBUF state ----
        wqkv_sb = singles.tile([128, NKB, NSLAB * DH], BF16)
        wo_sb = singles.tile([128, HQ, D], BF16)
        cos_sb = singles.tile([128, S], BF16)
        sin_sb = singles.tile([128, S], BF16)
        ident = singles.tile([128, 128], BF16)
        ones_sb = singles.tile([128, 128], BF16)
        maskT = singles.tile([128, 128], F32)
        # q slabs (m=0..3) + k slab (m=4), dh-major [dh, S], RoPE'd
        qkT_sb = singles.tile([128, HQ + 1, S], BF16)
        # V seq-major: tile t = rows [128t..128t+128) x [dh 128]
        v_sb = singles.tile([128, NQB, DH], BF16)
        # ctx^T per q-head slab [dh, S], softmax-normalized
        ctxT_sb = singles.tile([128, HQ, S], BF16)

        make_identity(nc, ident)
        nc.vector.memset(ones_sb, 1.0)
        # maskT[kv, q] = 0 where q >= kv (valid, diag incl), else -1e9
        nc.gpsimd.memset(maskT, 0.0)
        nc.gpsimd.affine_select(
            out=maskT,
            in_=maskT,
            compare_op=mybir.AluOpType.is_ge,
            fill=-1e9,
            base=0,
            pattern=[[1, 128]],
            channel_multiplier=-1,
        )

        with (
            tc.tile_pool(name="hidp", bufs=44) as hidp,
            tc.tile_pool(name="ropep", bufs=4) as ropep,
            tc.tile_pool(name="vtmp", bufs=2) as vtmpp,
            tc.tile_pool(name="ptp", bufs=4) as ptp,
            tc.tile_pool(name="paccp", bufs=2) as paccp,
            tc.tile_pool(name="linvp", bufs=2) as linvp,
            tc.tile_pool(name="outsb", bufs=4) as osp,
            # PSUM budget: 3 (mm: proj slabs + sT) + 1 (l) + 2 (ctx) + 2
            # (out: o_proj + v-transpose) = 8 banks
            tc.tile_pool(name="ps_mm", bufs=3, space="PSUM") as mmp,
            tc.tile_pool(name="ps_l", bufs=1, space="PSUM") as lpp,
            tc.tile_pool(name="ps_ctx", bufs=2, space="PSUM") as cpp,
            tc.tile_pool(name="ps_out", bufs=2, space="PSUM") as opp,
        ):
            for n in range(NCHUNK):
                # ---- phase 1: qkv projection for seq chunk n ----
                ht = []
                for kb in range(NKB):
                    if n == 0:
                        # full weight rows, interleaved so the first matmuls
                        # only wait for the (kb=0) pair
                        nc.sync.dma_start(
                            out=wqkv_sb[:, kb, :],
                            in_=wqkv[kb * 128:(kb + 1) * 128, :],
                        )
                    t_ = hidp.tile([128, 512], BF16, name=f"ht_{n}_{kb}", tag="ht")
                    nc.sync.dma_start(out=t_, in_=hidT[n, kb])
                    ht.append(t_)
                    if n == 0 and kb == 1:
                        nc.sync.dma_start(out=cos_sb, in_=cos_e[:])
                        nc.sync.dma_start(out=sin_sb, in_=sin_e[:])
                sl = slice(n * 512, (n + 1) * 512)
                # k slab first so its RoPE is long done when attention starts;
                # (slab_idx in qkT_sb/v, column offset in wqkv)
                SLABS = ((4, 512), (0, 0), (1, 128), (2, 256), (3, 384), (5, 640))
                if n == 0:
                    # chunk 0 rides the initial DMA wave: kb-inner over all 6
                    # slabs so each arriving ht tile unlocks 6 matmuls instead
                    # of racing one slab ahead of the DMA stream. Attention
                    # pools are idle until chunk 0 drains, so borrow their
                    # PSUM banks for the extra 3 accumulators.
                    groups = [SLABS]
                    slab_pools = [mmp, mmp, mmp, lpp, cpp, opp]
                    slab_tags = ["mm", "mm", "mm", "l", "ctx", "out"]
                else:
                    groups = [(s,) for s in SLABS]
                    slab_pools = [mmp] * 6
                    slab_tags = ["mm"] * 6
                vt_box = [None]

                def finish_slab(m, ps):
                    if m < NSLAB - 1:
                        # q or k slab: copy out of PSUM, then RoPE in place
                        slab = qkT_sb[:, m, sl]
                        nc.scalar.copy(slab, ps)
                        rot = ropep.tile([128, 512], BF16, name="rot", tag="rot")
                        nc.sync.dma_start(out=rot[0:64, :], in_=qkT_sb[64:128, m, sl])
                        nc.sync.dma_start(out=rot[64:128, :], in_=qkT_sb[0:64, m, sl])
                        rt = ropep.tile([128, 512], BF16, name="rt", tag="rt")
                        nc.vector.tensor_mul(rt, rot, sin_sb[:, sl])
                        nc.vector.tensor_mul(slab, slab, cos_sb[:, sl])
                        nc.vector.tensor_add(slab, slab, rt)
                    else:
                        # v slab: stage to SBUF; PE-transposed to seq-major
                        # inside the attention A-loop below (keeps PE dense)
                        vt_box[0] = vtmpp.tile([128, 512], BF16, name="vt", tag="vt")
                        nc.scalar.copy(vt_box[0], ps)

                si = 0
                for grp in groups:
                    pss = []
                    for m, _ in grp:
                        pss.append(
                            slab_pools[si].tile(
                                [128, 512], F32, name=f"proj_ps_{n}_{m}",
                                tag=slab_tags[si],
                            )
                        )
                        si += 1
                    for kb in range(NKB):
                        for gi, (m, coff) in enumerate(grp):
                            nc.tensor.matmul(
                                pss[gi],
                                wqkv_sb[:, kb, coff:coff + 128],
                                ht[kb],
                                start=(kb == 0),
                                stop=(kb == NKB - 1),
                            )
                    for gi, (m, coff) in enumerate(grp):
                        finish_slab(m, pss[gi])
                vt = vt_box[0]
                if n == 0:
                    # o_proj weights: first needed after attention chunk 0
                    for h in range(HQ):
                        nc.sync.dma_start(
                            out=wo_sb[:, h, :], in_=wo[h * 128:(h + 1) * 128, :]
                        )

                # ---- phase 2: attention for q-chunk c = n ----
                c = n
                ntile = 4 * (c + 1)
                for h in range(HQ):
                    # (t, qoff, w): kv tile t covers seq [128t, 128t+128); for
                    # diagonal tiles only q >= 128t attends -> stream suffix
                    tiles = []
                    for t in range(ntile):
                        qoff = max(0, 128 * (t - 4 * c))
                        tiles.append((t, qoff, 512 - qoff))
                    nt = len(tiles)
                    l_ps = lpp.tile([128, 512], F32, name=f"l_ps_{c}_{h}", tag="l")
                    ctx_ps = cpp.tile(
                        [128, 512], F32, name=f"ctx_ps_{c}_{h}", tag="ctx"
                    )
                    # big chunks: sum pT tiles on DVE (one ones-matmul instead
                    # of one per tile, ~22us less PE stream); small chunks keep
                    # the per-tile ones-matmul (DVE is co-critical there)
                    if c >= 2:
                        pacc = paccp.tile(
                            [128, 512], BF16, name=f"pacc_{c}_{h}", tag="pacc"
                        )
                        nc.vector.memset(pacc, 0.0)
                    else:
                        pacc = None
                    pT_tiles = {}

                    def stage_a(i):
                        t, qoff, w = tiles[i]
                        sT = mmp.tile(
                            [128, 512], F32, name=f"sT_{c}_{h}_{t}", tag="mm"
                        )
                        nc.tensor.matmul(
                            sT[:, 0:w],
                            qkT_sb[:, HQ, t * 128:(t + 1) * 128],
                            qkT_sb[:, h, c * 512 + qoff:(c + 1) * 512],
                            start=True,
                            stop=True,
                        )
                        if t >= 4 * c:
                            # diagonal tile: first 128 streamed q's need mask
                            nc.vector.tensor_add(sT[:, 0:128], sT[:, 0:128], maskT)
                        pT = ptp.tile([128, 512], BF16, name=f"pT_{c}_{h}_{t}", tag="pt")
                        nc.scalar.activation(
                            pT[:, 0:w],
                            sT[:, 0:w],
                            mybir.ActivationFunctionType.Exp,
                            scale=SCALE,
                        )
                        pT_tiles[i] = pT
                        if pacc is not None:
                            # accumulate sum-over-tiles on DVE; the single
                            # all-ones matmul below then reduces over kv
                            nc.vector.tensor_add(
                                pacc[:, qoff:512], pacc[:, qoff:512], pT[:, 0:w]
                            )

                    def stage_c(i):
                        t, qoff, w = tiles[i]
                        pT = pT_tiles.pop(i)
                        first, last = i == 0, i == nt - 1
                        if pacc is None:
                            nc.tensor.matmul(
                                l_ps[:, qoff:512],
                                ones_sb,
                                pT[:, 0:w],
                                start=first,
                                stop=last,
                            )
                        nc.tensor.matmul(
                            ctx_ps[:, qoff:512],
                            v_sb[:, t, :],
                            pT[:, 0:w],
                            start=first,
                            stop=last,
                        )

                    DEPTH = 3
                    for i in range(nt):
                        stage_a(i)
                        if h == 0 and i < 4:
                            # chunk n's v tiles: dh-major -> seq-major, needed
                            # from stage_c(4c) onward
                            vtp = opp.tile([128, 128], BF16, name="vtp", tag="out")
                            nc.tensor.transpose(
                                vtp, vt[:, i * 128:(i + 1) * 128], ident
                            )
                            nc.vector.tensor_copy(v_sb[:, 4 * c + i, :], vtp)
                        if i >= DEPTH:
                            stage_c(i - DEPTH)
                    for i in range(max(0, nt - DEPTH), nt):
                        stage_c(i)
                    if pacc is not None:
                        nc.tensor.matmul(
                            l_ps, ones_sb, pacc, start=True, stop=True
                        )

                    # softmax normalize: 1/l = exp(-ln(l)) via two scalar-engine
                    # table activations (DVE reciprocal costs 6.5ns/elem and
                    # divide is not a DVE op), then one DVE multiply
                    lnl = linvp.tile([128, 512], F32, name=f"lnl_{c}_{h}", tag="lnl")
                    nc.scalar.activation(
                        lnl, l_ps, mybir.ActivationFunctionType.Ln
                    )
                    linv = linvp.tile(
                        [128, 512], F32, name=f"linv_{c}_{h}", tag="linv"
                    )
                    nc.scalar.activation(
                        linv, lnl, mybir.ActivationFunctionType.Exp, scale=-1.0
                    )
                    nc.vector.tensor_mul(
                        ctxT_sb[:, h, c * 512:(c + 1) * 512], ctx_ps, linv
                    )

                # ---- phase 3: o_proj, one chunk behind attention ----
                # Emitting chunk c-1 here (after attention c) keeps the next
                # chunk's RoPE chain ahead of the o_proj PSUM-drain copies in
                # the scalar/vector queues, so attention never stalls on them.
                oproj_chunks = [c - 1] if c < NCHUNK - 1 else [c - 1, c]
                for oc in oproj_chunks:
                    if oc < 0:
                        continue
                    for iq in range(4):
                        qb = 4 * oc + iq
                        for dc in range(8):
                            out_ps = opp.tile(
                                [128, 512], F32, name=f"out_ps_{qb}_{dc}", tag="out"
                            )
                            for h in range(HQ):
                                nc.tensor.matmul(
                                    out_ps,
                                    ctxT_sb[:, h, qb * 128:(qb + 1) * 128],
                                    wo_sb[:, h, dc * 512:(dc + 1) * 512],
                                    start=(h == 0),
                                    stop=(h == HQ - 1),
                                )
                            out_sb = osp.tile(
                                [128, 512], BF16, name="out_sb", tag="out_sb"
                            )
                            if dc % 2 == 0:
                                nc.scalar.copy(out_sb, out_ps)
                            else:
                                nc.vector.tensor_copy(out_sb, out_ps)
                            nc.sync.dma_start(out=out[qb, dc], in_=out_sb)

    return nc


def _legalize_waits(j):
    """Split multi-wait instructions: the TPB ISA gives each instruction (and
    each dynamic-DMA descriptor) a single semaphore-wait slot, and this walrus
    build errors on extras instead of splitting them. Hoist all but one wait
    into standalone EventSemaphore instructions on the issuing engine, placed
    immediately before the instruction (engine streams execute in program
    order, so the waits complete before the op issues / the descriptor posts).
    """
    n_new = 0
    for fn in j["functions"]:
        for bb in fn["blocks"]:
            insts = bb.get("instructions", [])
            out = []
            for inst in insts:
                si = inst.get("sync_info") or {}
                waits = si.get("on_wait") or []
                if len(waits) > 1:
                    for w in waits[:-1]:
                        n_new += 1
                        out.append(
                            {
                                "name": f"{inst['name']}-lw{n_new}",
                                "opcode": "EventSemaphore",
                                "engine": inst["engine"],
                                "ins": [],
                                "outs": [],
                                "debug": inst.get("debug"),
                                "sync_info": {"on_update": [], "on_wait": [w]},
                            }
                        )
                    si = dict(si)
                    si["on_wait"] = [waits[-1]]
                    inst = dict(inst)
                    inst["sync_info"] = si
                out.append(inst)
            bb["instructions"] = out
    return j


def _patch_json(nc):
    import json

    orig = nc.to_json_bytes

    def patched():
        j = json.loads(orig())
        return json.dumps(_legalize_waits(j)).encode()

    nc.to_json_bytes = patched
    return nc


_NC_CACHE = None


def _get_nc():
    global _NC_CACHE
    if _NC_CACHE is None:
        _NC_CACHE = _patch_json(build_kernel())
    return _NC_CACHE


def _prep_in_maps(hidden_states, W_qkv, W_o):
    hid = np.asarray(hidden_states, dtype=np.float32).reshape(S, D)
    # packed [chunk, kb, 128, 512]: hidT[kb*128+p, n*512+c]
    hidT = np.ascontiguousarray(
        hid.T.reshape(NKB, 128, NCHUNK, 512).transpose(2, 0, 1, 3)
    ).astype(np_bf16)
    W_qkv = np.asarray(W_qkv, dtype=np.float32)
    W_o = np.asarray(W_o, dtype=np.float32)

    inv = 1.0 / (THETA ** (np.arange(0, DH, 2, dtype=np.float64) / DH))
    fr = np.arange(S, dtype=np.float64)[:, None] * inv[None, :]  # [S, 64]
    cosT = np.cos(fr).T
    sinT = np.sin(fr).T
    cos2 = np.ascontiguousarray(np.concatenate([cosT, cosT], 0)).astype(np_bf16)
    sinS = np.ascontiguousarray(np.concatenate([-sinT, sinT], 0)).astype(np_bf16)

    in_maps = []
    for i in range(NCORES):
        q_cols = W_qkv[:, 512 * i:512 * i + 512]
        k_cols = W_qkv[:, H * DH + 128 * i:H * DH + 128 * i + 128]
        v_cols = W_qkv[:, (H + HKV) * DH + 128 * i:(H + HKV) * DH + 128 * i + 128]
        wqkv_i = np.ascontiguousarray(
            np.concatenate([q_cols, k_cols, v_cols], axis=1)
        ).astype(np_bf16)
        wo_i = np.ascontiguousarray(W_o[512 * i:512 * i + 512, :]).astype(np_bf16)
        in_maps.append(
            {
                "hidT": hidT,
                "wqkv": wqkv_i,
                "wo": wo_i,
                "cos2": cos2,
                "sinS": sinS,
            }
        )
    return in_maps


def _run(in_maps, trace=False, **kw):
    nc = _get_nc()
    return run_bass_kernel_spmd(
        nc, in_maps, core_ids=list(range(NCORES)), trace=trace, **kw
    )


def _gather(res):
    total = np.zeros((S, D), dtype=np.float32)
    for i in range(NCORES):
        part = np.asarray(res.results[i]["out"], dtype=np.float32)
        # unpack [qb, dc, 128, 512] -> [S, D]
        total += part.transpose(0, 2, 1, 3).reshape(S, D)
    return total.reshape(S, B, D).astype(np.float32)


def kernel(hidden_states, sequence_mask, W_qkv, W_o):
    in_maps = _prep_in_maps(hidden_states, W_qkv, W_o)
    return _gather(_run(in_maps))
